# revision 20
# baseline (speedup 1.0000x reference)
"""Chamfer distance kernel for Trainium2 (8 NeuronCores).

Inputs: xyz1, xyz2: [4, 8192, 3] f32. Outputs (dist1, dist2, idx1, idx2):
squared nearest-neighbor distances and int32 argmin indices in both
directions per batch, matching the fp32 reference exactly.

Sharding: 8 cores = 4 batches x 2 directions. Core 2b computes
xyz1[b]->xyz2[b] (dist1/idx1), core 2b+1 computes xyz2[b]->xyz1[b]
(dist2/idx2). Each core brute-forces 8192 queries x 8192 targets.

Per-core device algorithm (v2 — dual-engine PSUM drain):
  * Host precomputes a K=30 bf16 "split lift" such that
      sum_k QL[k,m] * TL[k,n]  ~=  -d(q_m, t_n)
    to ~2e-5 abs error (5-term distance lift, 3-way bf16 splits, 6
    largest cross blocks stacked along contraction).
  * PE: per 128-query row-tile (64 tiles), 16 matmuls [30,128]^T x
    [30,512] -> 16 chunks of -d in 4 PSUM groups G0..G3 (G0,G2 in
    banks 0-3; G1,G3 in banks 4-7, double buffered).
  * A DVE/vector instruction may read only ONE input from PSUM, so the
    drain is split: ACT (scalar) casts 10 chunks (G0, G2, G3 banks
    0-1) to bf16 SBUF; DVE evacuates the other 6 with tensor_reduce
    max over the bank axis (R1a=G1b01, R1b=G1b23, R3=G3b23 -> 512-wide
    each; single PSUM operand, no dependency on the ACT casts), then 4
    all-SBUF bf16 merges in scalar_tensor_tensor form (DVE 4x_2p perf
    mode, 0.25 cyc/elem). PE wave order G1,G0,G2,G3 with per-bank-pair
    gating keeps every cross-tile dependency cycle under the pipeline
    period.
  * Output per query: 3072 lanes. Lanes [0,2048) = max(G0,G2): targets
    {l, l+4096}; [2048,2560) = max(R1a,R3): {2048,2560,7168,7680}+s;
    [2560,3072) = max(R1b, fold(c3)): {3072,3584,6144,6656}+s.
  * Per tile the [128, 3072] bf16 folded lanes are DMA'd to HBM from
    a 10-slot SBUF ring (overlapped with compute, 48 MB/core total).
  * Engine balance per tile: ACT ~4.82us, DVE ~4.75us, PE ~3.41us.

Host post-pass: lanes hold max of -d~ over their (static) candidate
sets. Top-8 lanes per query (argpartition) -> <=32 candidate indices,
exact fp32 rerank (bit-identical op order to the XLA CPU reference),
min with smallest-index tie-break. A near-tie detector recomputes any
query with >=4 candidates inside the bf16 comparison window by exact
brute force (fires ~never on N(0,1) data, keeps clustered inputs
exact).
"""
import contextlib

import numpy as np
import ml_dtypes

import concourse.bass as bass
import concourse.mybir as mybir
from concourse.bass_utils import run_bass_kernel_spmd

B = 4             # batches
N = 8192          # queries per core (= points per cloud)
M = 8192          # targets per core
CH = 512          # matmul chunk (free dim; one PSUM bank)
NCH = M // CH     # 16 chunks per row-tile
RT = 128          # queries per row-tile
NRT = N // RT     # 64 row-tiles
KLIFT = 30        # 6 split blocks x 5 lift rows
GRP = 4           # chunks per PSUM group (4 banks)
NGRP = NCH // GRP
NLANE = 3328      # folded lanes per query shipped to host
LRING = 12        # lanes SBUF ring slots (overlapping output DMAs)

# on-device stopwatch calibration (gpsimd nop quantum), used by test.py
TIMER_QUANTUM_NS = 51457.0 / 60000.0  # ns per pool nop cycle (calibrated)
TIMER_NOP = 12000                     # pool cycles per watcher tick (~10.3us)

_NC_CACHE = {}


def _build_candmap():
    """[NLANE, 4] uint32: the (static) target-candidate set of each lane.

    Padding entries are M (out of range -> inf distance in the rerank).
    Layout: [0,2048) max(G0,G2); [2048,2304) quads of chunks 4,5;
    [2304,2560) quads of 6,7; [2560,3072) chunk 12 vs 13;
    [3072,3328) quads of 14,15.
    """
    cm = np.full((NLANE, 4), M, np.uint32)
    l = np.arange(2048, dtype=np.uint32)
    cm[0:2048, 0] = l
    cm[0:2048, 1] = l + 4096
    m = np.arange(256, dtype=np.uint32)
    e = np.arange(4, dtype=np.uint32)
    for base, c0 in ((2048, 4), (2304, 6), (3072, 14)):
        t0 = 512 * (c0 + m // 128) + 4 * (m % 128)
        cm[base:base + 256] = t0[:, None] + e[None, :]
    s = np.arange(512, dtype=np.uint32)
    cm[2560:3072, 0] = 6144 + s
    cm[2560:3072, 1] = 6656 + s
    return cm


_CANDMAP = _build_candmap()


def _gen_kernel(repeat=1, timer_ticks=0, out_lanes=NLANE):
    """Build the per-core bass program.

    repeat > 1 replays the whole compute (benchmarking).
    timer_ticks > 0 adds a gpsimd tick counter; output "tns" holds the
    tick count at compute completion (on-device stopwatch).
    """
    nc = bass.Bass()
    qlift = nc.dram_tensor("qlift", [KLIFT, N], mybir.dt.bfloat16, kind="ExternalInput")
    tlift = nc.dram_tensor("tlift", [KLIFT, M], mybir.dt.bfloat16, kind="ExternalInput")
    lanes_out = nc.dram_tensor("lanes", [RT, NRT * NLANE], mybir.dt.bfloat16,
                               kind="ExternalOutput")
    if timer_ticks:
        tns_out = nc.dram_tensor("tns", [1, 2], mybir.dt.int32, kind="ExternalOutput")

    mx = mybir.AluOpType.max
    mul = mybir.AluOpType.mult
    ax_x = mybir.AxisListType.X
    NT = NRT * repeat

    with (
        nc.sbuf_tensor([KLIFT, N], mybir.dt.bfloat16) as ql_sb,
        nc.sbuf_tensor([KLIFT, M], mybir.dt.bfloat16) as tl_sb,
        nc.sbuf_tensor([RT, 2, GRP * CH], mybir.dt.bfloat16) as c0sb,  # cast G0
        nc.sbuf_tensor([RT, 2, GRP * CH], mybir.dt.bfloat16) as c2sb,  # cast G2
        nc.sbuf_tensor([RT, 2, 2 * CH], mybir.dt.bfloat16) as c3sb,    # cast G3b01
        nc.sbuf_tensor([RT, 3, CH], mybir.dt.bfloat16) as rsb,         # reduces
        nc.sbuf_tensor([RT, CH], mybir.dt.bfloat16) as cfold,          # fold(c3)
        nc.sbuf_tensor([RT, LRING, NLANE], mybir.dt.bfloat16) as lanes_sb,
        nc.sbuf_tensor([1, 2], mybir.dt.int32) as cnt_sb,
        nc.semaphore() as s_in,
        nc.semaphore() as s_mm,
        nc.semaphore() as s_cast,
        nc.semaphore() as s_ev,
        nc.semaphore() as s_tile,
        nc.semaphore() as s_out,
        nc.semaphore() as s_fin,
        contextlib.ExitStack() as st,
    ):
        psall = st.enter_context(
            nc.psum_tensor("psall", [RT, 2 * GRP, CH], mybir.dt.float32))
        ps = [psall[:, 0:GRP, :], psall[:, GRP:2 * GRP, :]]
        HW = 2 * CH  # 1024: half a PSUM group
        # PE wave order per tile: G1(ps1), G0(ps0), G2(ps0), G3(ps1).
        # Chunks: G0=0-3, G1=4-7, G2=8-11, G3=12-15; matmul order is
        # 4,5,6,7, 0,1,2,3, 8,9,10,11, 12,13,14,15.
        with nc.Block() as block:

            @block.sync
            def _(sync):
                sync.dma_start(ql_sb[:, :], qlift[:, :]).then_inc(s_in, 16)
                sync.dma_start(tl_sb[:, :], tlift[:, :]).then_inc(s_in, 16)
                for gi in range(NT):
                    i = gi % NRT
                    sync.wait_ge(s_ev, 3 * gi + 3)
                    sync.dma_start(
                        lanes_out[:, i * NLANE:i * NLANE + out_lanes],
                        lanes_sb[:, gi % LRING, 0:out_lanes],
                    ).then_inc(s_out, 16)
                sync.wait_ge(s_out, 16 * NT)
                if timer_ticks:
                    sync.dma_start(tns_out[:, :], cnt_sb[:, :]).then_inc(s_fin, 16)
                    sync.wait_ge(s_fin, 16)

            if timer_ticks:
                @block.gpsimd
                def _(gpsimd):
                    gpsimd.wait_ge(s_in, 32)
                    with gpsimd.register("tk") as tk:
                        gpsimd.reg_mov(tk, 0)
                        for _ in range(timer_ticks):
                            gpsimd.nop(cycle_cnt=TIMER_NOP)
                            gpsimd.reg_add(tk, tk, 1)
                            gpsimd.reg_save(cnt_sb[0:1, 0:1], tk)

            @block.tensor
            def _(tensor):
                tensor.wait_ge(s_in, 32)
                for gi in range(NT):
                    i = gi % NRT
                    lhsT = ql_sb[:, i * RT:(i + 1) * RT]

                    def mm(c, t, k):
                        tensor.matmul(
                            ps[t][:, k, :], lhsT,
                            tl_sb[:, c * CH:(c + 1) * CH],
                            start=True, stop=True,
                        ).then_inc(s_mm, 1)

                    # G1 -> ps1: b01 freed by c3(gi-1), b23 by R3(gi-1)
                    if gi >= 1:
                        tensor.wait_ge(s_cast, 3 * gi)
                    mm(4, 1, 0)
                    mm(5, 1, 1)
                    if gi >= 1:
                        tensor.wait_ge(s_ev, 3 * gi)
                    mm(6, 1, 2)
                    mm(7, 1, 3)
                    # G0 -> ps0: freed by C2(gi-1)
                    if gi >= 1:
                        tensor.wait_ge(s_cast, 3 * gi - 1)
                    for k in range(GRP):
                        mm(k, 0, k)
                    # G2 -> ps0: freed by C0(gi)
                    tensor.wait_ge(s_cast, 3 * gi + 1)
                    for k in range(GRP):
                        mm(8 + k, 0, k)
                    # G3 -> ps1: b01 freed by R1a(gi), b23 by R1b(gi)
                    tensor.wait_ge(s_ev, 3 * gi + 1)
                    mm(12, 1, 0)
                    mm(13, 1, 1)
                    tensor.wait_ge(s_ev, 3 * gi + 2)
                    mm(14, 1, 2)
                    mm(15, 1, 3)

            @block.scalar
            def _(scalar):
                for gi in range(NT):
                    slot = gi % 2
                    if gi >= 2:
                        scalar.wait_ge(s_ev, 3 * gi - 3)  # cast-slot reuse
                    scalar.wait_ge(s_mm, 16 * gi + 8)
                    scalar.copy(
                        c0sb[:, slot, :],
                        ps[0][:, :, :].rearrange("p a b -> p (a b)"),
                    ).then_inc(s_cast, 1)
                    scalar.wait_ge(s_mm, 16 * gi + 12)
                    scalar.copy(
                        c2sb[:, slot, :],
                        ps[0][:, :, :].rearrange("p a b -> p (a b)"),
                    ).then_inc(s_cast, 1)
                    scalar.wait_ge(s_mm, 16 * gi + 14)
                    scalar.copy(
                        c3sb[:, slot, :],
                        ps[1][:, 0:2, :].rearrange("p a b -> p (a b)"),
                    ).then_inc(s_cast, 1)

            @block.vector
            def _(vector):
                QD = CH // 2  # 256: quad-reduce output width per bank pair
                for gi in range(NT):
                    slot = gi % 2
                    lane = lanes_sb[:, gi % LRING, :]
                    if gi >= LRING:
                        vector.wait_ge(s_out, 16 * (gi - (LRING - 1)))
                    # quad reduces: max over 4 adjacent columns (packed
                    # inner axis -> full-rate streaming out of PSUM)
                    # R1a = quads of G1 banks 0,1 (chunks 4,5)
                    vector.wait_ge(s_mm, 16 * gi + 2)
                    vector.tensor_reduce(
                        lane[:, 2048:2048 + QD],
                        ps[1][:, 0:2, :].rearrange("p a (b c) -> p (a b) c", c=4),
                        ax_x, mx).then_inc(s_ev, 1)
                    # R1b = quads of G1 banks 2,3 (chunks 6,7)
                    vector.wait_ge(s_mm, 16 * gi + 4)
                    vector.tensor_reduce(
                        lane[:, 2048 + QD:2048 + 2 * QD],
                        ps[1][:, 2:4, :].rearrange("p a (b c) -> p (a b) c", c=4),
                        ax_x, mx).then_inc(s_ev, 1)
                    # sliceA = max(C0, C2) -> lanes[0:2048]
                    vector.wait_ge(s_cast, 3 * gi + 2)
                    vector.scalar_tensor_tensor(
                        lane[:, 0:GRP * CH], c0sb[:, slot, :], 1.0,
                        c2sb[:, slot, :], mul, mx)
                    # cfold = fold(c3): chunk 12 vs 13 -> lanes[2560:3072]
                    vector.wait_ge(s_cast, 3 * gi + 3)
                    vector.scalar_tensor_tensor(
                        lane[:, 2048 + 2 * QD:2048 + 2 * QD + CH],
                        c3sb[:, slot, 0:CH], 1.0,
                        c3sb[:, slot, CH:2 * CH], mul, mx)
                    # R3h = quads of G3 banks 2,3 (chunks 14,15)
                    vector.wait_ge(s_mm, 16 * gi + 16)
                    vector.tensor_reduce(
                        lane[:, 2048 + 2 * QD + CH:NLANE],
                        ps[1][:, 2:4, :].rearrange("p a (b c) -> p (a b) c", c=4),
                        ax_x, mx).then_inc(s_ev, 1)
    return nc


def _split3(a):
    """3-way bf16 split: a ~= h + m + l (each bf16)."""
    a = a.astype(np.float32)
    h = a.astype(ml_dtypes.bfloat16)
    r = a - h.astype(np.float32)
    m = r.astype(ml_dtypes.bfloat16)
    l = (r - m.astype(np.float32)).astype(ml_dtypes.bfloat16)
    return h, m, l


def _lift_q(Q):
    """[n,3] -> [5,n] f32 rows: 2qx, 2qy, 2qz, -|q|^2, 1."""
    n = Q.shape[0]
    return np.stack(
        [2 * Q[:, 0], 2 * Q[:, 1], 2 * Q[:, 2],
         -(Q * Q).sum(-1, dtype=np.float32), np.ones(n, np.float32)], 0
    ).astype(np.float32)


def _lift_t(T):
    n = T.shape[0]
    return np.stack(
        [T[:, 0], T[:, 1], T[:, 2], np.ones(n, np.float32),
         -(T * T).sum(-1, dtype=np.float32)], 0
    ).astype(np.float32)


def _split_lift(Lq, Lt):
    """K=30 bf16 operand pair whose inner product reproduces Lq.T @ Lt to
    ~2e-5: blocks (qh,th), (qh,tm), (qm,th), (qh,tl), (qm,tm), (ql,th)."""
    qh, qm, ql = _split3(Lq)
    th, tm, tl = _split3(Lt)
    QL = np.concatenate([qh, qh, qm, qh, qm, ql], 0)
    TL = np.concatenate([th, tm, th, tl, tm, th], 0)
    return np.ascontiguousarray(QL), np.ascontiguousarray(TL)


def _host_rerank(cand, Q, T):
    """Exact fp32 rerank of candidate target indices per query.

    cand: [n, K] uint32 candidate indices (out-of-range values allowed).
    Returns (dist [n] f32, idx [n] int32) matching fp32 argmin semantics
    (smallest index on exact ties).

    Queries whose candidates show >=4 near-ties inside the bf16
    comparison window (where the device fold could have dropped the true
    argmin) are recomputed by exact brute force. Fires ~never on N(0,1)
    clouds; keeps adversarial clustered/duplicated inputs exact.
    """
    n = cand.shape[0]
    ci = cand.astype(np.int64)
    invalid = ci >= T.shape[0]
    ci_safe = np.where(invalid, 0, ci)
    t = T[ci_safe]                      # [n, K, 3]
    q = Q[:, None, :]
    dx = q[..., 0] - t[..., 0]
    dy = q[..., 1] - t[..., 1]
    dz = q[..., 2] - t[..., 2]
    d = (dx * dx + dy * dy) + dz * dz   # exact f32, same op order as reference
    d = np.where(invalid, np.float32(np.inf), d)
    order = np.lexsort((ci_safe, d), axis=-1)   # by (d, idx)
    k = order[:, 0]
    rows = np.arange(n)
    dist = d[rows, k].astype(np.float32)
    idx = ci_safe[rows, k].astype(np.int32)

    w = dist * np.float32(2 ** -6) + np.float32(1e-4) * np.maximum(dist, 1.0)
    near = (d <= (dist + w)[:, None]).sum(1)
    suspect = np.where(near >= 4)[0]
    for s0 in range(0, len(suspect), 256):
        rows_s = suspect[s0:s0 + 256]
        qd = Q[rows_s][:, None, :] - T[None, :, :]
        sq = qd * qd
        dd = (sq[..., 0] + sq[..., 1]) + sq[..., 2]
        ii = np.argmin(dd, axis=1)
        idx[rows_s] = ii.astype(np.int32)
        dist[rows_s] = dd[np.arange(len(rows_s)), ii]
    return dist, idx


def kernel(xyz1, xyz2):
    xyz1 = np.ascontiguousarray(np.asarray(xyz1, dtype=np.float32))
    xyz2 = np.ascontiguousarray(np.asarray(xyz2, dtype=np.float32))
    assert xyz1.shape == (B, N, 3) and xyz2.shape == (B, M, 3)

    if "nc" not in _NC_CACHE:
        _NC_CACHE["nc"] = _gen_kernel()
    nc = _NC_CACHE["nc"]

    # per-core inputs: core 2b -> (Q=xyz1[b], T=xyz2[b]); core 2b+1 swapped
    in_maps = []
    QT = []
    for b in range(B):
        for d in range(2):
            Q, T = (xyz1[b], xyz2[b]) if d == 0 else (xyz2[b], xyz1[b])
            QL, TL = _split_lift(_lift_q(Q), _lift_t(T))
            in_maps.append({"qlift": QL.astype(ml_dtypes.bfloat16),
                            "tlift": TL.astype(ml_dtypes.bfloat16)})
            QT.append((Q, T))

    # Retry a couple of times: the axon-tunneled devices occasionally come
    # back NRT_EXEC_UNIT_UNRECOVERABLE after an earlier aborted session and
    # recover on a later attempt.
    last_exc = None
    for attempt in range(3):
        try:
            res = run_bass_kernel_spmd(nc, in_maps, core_ids=list(range(8)))
            break
        except Exception as e:  # noqa: BLE001
            last_exc = e
            if attempt < 2:
                import time as _time
                _time.sleep(15 * (attempt + 1))
    else:
        raise last_exc

    dist1 = np.empty((B, N), np.float32)
    dist2 = np.empty((B, M), np.float32)
    idx1 = np.empty((B, N), np.int32)
    idx2 = np.empty((B, M), np.int32)
    for core in range(8):
        b, d = divmod(core, 2)
        r = res.results[core]
        # [128, 64*3072] -> [8192, 3072]: query g = i*128 + p
        lanes = (r["lanes"].reshape(RT, NRT, NLANE).transpose(1, 0, 2)
                 .reshape(N, NLANE).astype(np.float32))
        top8 = np.argpartition(lanes, NLANE - 8, axis=1)[:, NLANE - 8:]
        # expand each winning lane to its static candidate set
        cand = _CANDMAP[top8].reshape(N, 32)
        Q, T = QT[core]
        dist, idx = _host_rerank(cand, Q, T)
        if d == 0:
            dist1[b], idx1[b] = dist, idx
        else:
            dist2[b], idx2[b] = dist, idx
    return dist1, dist2, idx1, idx2
